# revision 24
# baseline (speedup 1.0000x reference)
"""BinConv3d (sign-binarized 3x3x3 conv, NCDHW) on 8 Trainium2 NeuronCores.

Full inputs in, full output out. Sharding: depth dim D=64 split 8 ways
(8 output planes per core) with a 1-plane halo on the input; conv weights
and bias replicated to every core.

Host prep: each core's input slab is rearranged to [plane, quarter, ci,
34, 130] fp32 — the H dim split into 4 quarter-row panels of 32 rows,
each padded with 1 halo row above/below and 1 zero col left/right, so
the device needs no data reshuffling at all.

Per-core kernel (Bass/Tile):
  - One [128, 34*130] DMA per plane (partition = quarter*32 + ci), then
    ScalarE Sign fp32 -> bf16 (zero pads stay zero).
  - Conv = 27 accumulating matmuls (K=32 ci, M=64 co, N=512) per 4-row
    output tile; every (kd, kh, kw) tap is a free-dim AP offset.
  - 16-way PE tiling: quarter q runs on PE row-group q (tile_position
    row 32q); even/odd 4-row blocks run on PE column halves. 8 matmuls
    issue back-to-back per tap and run concurrently: full 128x128 array.
  - PSUM: 4 banks per generation (bank = quarter, partitions 0-63 even
    block / 64-127 odd block), double-buffered = all 8 banks.
  - PSUM drained with bias add on ScalarE (even) / VectorE (odd) into a
    [128, 2048] staging tile, flushed to HBM as 2x512KB DMAs on
    complementary SBUF port sets.
"""

import numpy as np
import ml_dtypes

import concourse.bass as bass
import concourse.mybir as mybir
import concourse.tile as tile
from concourse import bacc
from concourse.bass import ts
from concourse.bass_utils import run_bass_kernel_spmd
from concourse.tile_rust import add_dep_helper

CI = 32
CO = 64
D_FULL = 64
N_CORES = 8
D_OUT = D_FULL // N_CORES  # output planes per core
D_IN = D_OUT + 2  # input planes per core (1-plane halo each side)

_cache = {}


def build_conv_program(n_in_planes=D_IN, n_out_planes=D_OUT, H=128, W=128,
                       debug=False):
    """Build the per-core Bass program (SPMD: same program on all cores)."""
    f32 = mybir.dt.float32
    bf16 = mybir.dt.bfloat16
    Hq = H // 4          # rows per quarter-panel
    Hqp, Wp = Hq + 2, W + 2
    n_pairs = Hq // 8    # even/odd block pairs per quarter
    assert Hq % 8 == 0 and W == 128

    nc = bacc.Bacc("TRN2", target_bir_lowering=False, debug=debug)
    x_in = nc.declare_dram_parameter(
        "xs", [n_in_planes, 4, CI, Hqp, Wp], f32, isOutput=False)
    w_in = nc.declare_dram_parameter("wst", [128, 27, 2 * CO], bf16,
                                     isOutput=False)
    b_in = nc.declare_dram_parameter("bias", [128, 1], f32, isOutput=False)
    y_out = nc.declare_dram_parameter("y", [CO, n_out_planes, H, W], f32,
                                      isOutput=True)

    with tile.TileContext(nc) as tc:
        with (
            tc.tile_pool(name="const", bufs=1) as constp,
            tc.tile_pool(name="raw", bufs=2) as rawp,
            tc.tile_pool(name="sgn", bufs=5) as sgnp,
            tc.tile_pool(name="stg", bufs=4) as stgp,
            tc.tile_pool(name="psum", bufs=2, space="PSUM") as psump,
        ):
            wt = constp.tile([128, 27, 2 * CO], bf16)
            nc.sync.dma_start(out=wt[:], in_=w_in[:])
            bs = constp.tile([128, 1], f32)
            nc.sync.dma_start(out=bs[:], in_=b_in[:])

            sgns = {}

            def load_plane(p):
                # loads ride the ACT HWDGE ring; stores use the SP ring, so
                # a load is never FIFO-queued behind a burst of stores
                raw = rawp.tile([128, Hqp, Wp], f32, tag="raw")
                nc.scalar.dma_start(
                    out=raw[:],
                    in_=x_in[p].rearrange("q c h w -> (q c) h w"),
                )
                sgn = sgnp.tile([128, Hqp, Wp], bf16, tag="sgn")
                nc.scalar.sign(sgn[:], raw[:])
                sgns[p] = sgn

            for p in range(3):
                load_plane(p)

            for d in range(n_out_planes):
                if d + 3 < n_in_planes:
                    load_plane(d + 3)
                for pi in range(n_pairs):
                    # generation: for each quarter, blocks 2*pi (even,
                    # cols 0-63) and 2*pi+1 (odd, cols 64-127)
                    pts = [psump.tile([128, 512], f32, tag=f"pt{q}",
                                      name=f"pt{q}_{d}_{pi}")
                           for q in range(4)]
                    for tap in range(27):
                        kd, r = divmod(tap, 9)
                        kh, kw = divmod(r, 3)
                        sg = sgns[d + kd]
                        for q in range(4):
                            for half in range(2):
                                blk = 2 * pi + half
                                rhs = sg[32 * q:32 * q + 32,
                                         4 * blk + kh:4 * blk + kh + 4,
                                         kw:kw + W]
                                nc.tensor.matmul(
                                    pts[q][64 * half:64 * half + 64, :],
                                    lhsT=wt[32 * q:32 * q + 32, tap, 0:CO],
                                    rhs=rhs,
                                    start=(tap == 0),
                                    stop=(tap == 26),
                                    tile_position=(32 * q, 64 * half),
                                    skip_group_check=True,
                                )
                    # drain on DVE only: ScalarE is reserved for Sign so
                    # psum release is never stuck behind a sign burst
                    stg = stgp.tile([128, 4 * 512], f32, tag="stg")
                    for q in range(4):
                        for half in range(2):
                            sl = slice(64 * half, 64 * half + 64)
                            nc.vector.tensor_scalar_add(
                                out=stg[sl, ts(q, 512)], in0=pts[q][sl, :],
                                scalar1=bs[sl],
                            )
                    # out rows: quarter q -> 32q + 8*pi + 4*half .. +4
                    yv = y_out[:, d].rearrange("co (q hi) w -> co q hi w", q=4)
                    for half in range(2):
                        dst = yv[:, :, 8 * pi + 4 * half:8 * pi + 4 * half + 4,
                                 :].rearrange("co q hi w -> co q (hi w)")
                        src = stg[64 * half:64 * half + 64, :].rearrange(
                            "co (q n) -> co q n", q=4)
                        nc.sync.dma_start(out=dst, in_=src)

    nc.compile()
    _batch_matmul_sem_incs(nc)
    return nc


def _batch_matmul_sem_incs(nc):
    """Replace per-matmul semaphore increments with batched increments on
    threshold-crossing matmuls only.

    Concurrent MATMULs on the PE complete in pc order, so a single +N inc
    on matmul #k is equivalent to +1 incs on matmuls #1..#k for every
    waiter. Sem-write serialization costs ~26ns per inc on the PE, which
    at ~7000 matmuls dominates issue bandwidth. Only sems whose updaters
    are exclusively immediate-mode matmul incs are touched; every waited
    value (ge or eq, immediate only) is preserved exactly.
    """
    import bisect
    from collections import defaultdict

    f = nc.m.functions[0]
    insts = [i for blk in f.blocks for i in blk.instructions]
    updaters = defaultdict(list)  # sem id -> [(order, inst, value)]
    unsafe = set()  # sem ids we must not touch
    wait_vals = defaultdict(set)
    for inst in insts:
        si = inst.sync_info
        if si is None:
            continue
        for u in si.on_update:
            if u.sync_type != 'semaphore':
                continue
            if (isinstance(inst, mybir.InstMatmult)
                    and u.update_mode == 'sem-inc'
                    and u.update_reg is None):
                updaters[u.id].append((len(updaters[u.id]), inst, u.update_value))
            else:
                unsafe.add(u.id)
        for w in si.on_wait:
            if w.sync_type != 'semaphore':
                continue
            if w.wait_reg is not None or w.wait_value is None:
                unsafe.add(w.id)
            else:
                wait_vals[w.id].add(w.wait_value)

    n_removed = 0
    for sem, ups in updaters.items():
        if sem in unsafe or len(ups) < 16:
            continue
        if any(v != 1 for _, _, v in ups):
            continue
        total = len(ups)
        if any(v > total for v in wait_vals.get(sem, set())):
            continue
        # marks: the minimal updater indices that waiters actually need
        # (plus the final one, so sem totals seen by teardown code move
        # monotonically to a deterministic value)
        need = sorted(wait_vals.get(sem, set()) | {total})
        mark_ks = sorted({wv - 1 for wv in need})  # 0-based updater index
        # rank map: old threshold w -> 1 + index of first mark with
        # cumulative (= k+1) >= w
        cums = [k + 1 for k in mark_ks]
        mark_set = set(mark_ks)
        # strip +1 updates from unmarked matmuls (engine completes in pc
        # order, so the marked matmul's inc implies all earlier ones)
        for k, (_, inst, _) in enumerate(ups):
            if k in mark_set:
                continue
            si = inst.sync_info
            si.on_update = [u for u in si.on_update
                            if not (u.sync_type == 'semaphore'
                                    and u.id == sem)]
            inst.sync_info = si
            n_removed += 1
        # rewrite every wait on this sem into mark-rank space
        import bass_rust
        for inst in insts:
            si = inst.sync_info
            if si is None or not si.on_wait:
                continue
            changed = False
            lst = []
            for w in si.on_wait:
                if w.sync_type == 'semaphore' and w.id == sem:
                    nv = bisect.bisect_left(cums, w.wait_value) + 1
                    lst.append(bass_rust.SyncWait(
                        sync_type=w.sync_type, id=w.id, ant_name=w.ant_name,
                        wait_mode=w.wait_mode, wait_value=nv,
                        wait_reg=w.wait_reg))
                    changed = True
                else:
                    lst.append(w)
            if changed:
                si.on_wait = lst
                inst.sync_info = si
    return n_removed


def _get_program():
    if "nc" not in _cache:
        _cache["nc"] = build_conv_program()
    return _cache["nc"]


def prep_weights(W, b):
    W = np.asarray(W, dtype=np.float32)
    b = np.asarray(b, dtype=np.float32)
    # wst[q*32+ci, kd*9+kh*3+kw, half*64+co] = W[co, ci, kd, kh, kw],
    # replicated over the 4 row groups and the 2 col halves
    wq = W.transpose(1, 2, 3, 4, 0).reshape(CI, 27, CO)
    wq2 = np.concatenate([wq, wq], axis=2)  # duplicate col halves
    wst = np.ascontiguousarray(
        np.broadcast_to(wq2[None], (4, CI, 27, 2 * CO)).reshape(128, 27, 2 * CO)
    ).astype(ml_dtypes.bfloat16)
    bias = np.ascontiguousarray(
        np.concatenate([b, b]).reshape(128, 1).astype(np.float32))
    return wst, bias


def prep_x_slab(xpad, p_lo, n_planes, H=128, W=128):
    """xpad: [CI, D+2, H+2, W+2] zero-padded input. Returns
    [n_planes, 4, CI, H//4+2, W+2] fp32 slab for planes p_lo..p_lo+n_planes."""
    Hq = H // 4
    out = np.empty((n_planes, 4, CI, Hq + 2, W + 2), dtype=np.float32)
    for q in range(4):
        # padded rows 32q .. 32q+34 cover global rows 32q-1 .. 32q+33
        out[:, q] = xpad[:, p_lo:p_lo + n_planes,
                         Hq * q:Hq * q + Hq + 2, :].transpose(1, 0, 2, 3)
    return out


def _prep_inputs(x, W, b):
    x = np.asarray(x, dtype=np.float32)
    wst, bias = prep_weights(W, b)
    xpad = np.pad(x[0], ((0, 0), (1, 1), (1, 1), (1, 1)))
    in_maps = []
    for k in range(N_CORES):
        xs = prep_x_slab(xpad, D_OUT * k, D_IN)
        in_maps.append({"xs": xs, "wst": wst, "bias": bias})
    return in_maps


def run(x, W, b, trace=False):
    """Run the kernel; returns (output, BassKernelResults)."""
    nc = _get_program()
    in_maps = _prep_inputs(x, W, b)
    res = run_bass_kernel_spmd(nc, in_maps, list(range(N_CORES)), trace=trace)
    y = np.concatenate([res.results[k]["y"] for k in range(N_CORES)], axis=1)
    return y[None], res


def kernel(x, W, b):
    y, _ = run(x, W, b)
    return y


# revision 27
# speedup vs baseline: 1.0175x; 1.0175x over previous
"""BinConv3d (sign-binarized 3x3x3 conv, NCDHW) on 8 Trainium2 NeuronCores.

Full inputs in, full output out. Sharding: depth dim D=64 split 8 ways
(8 output planes per core) with a 1-plane halo on the input; conv weights
and bias replicated to every core.

Host prep: each core's input slab is rearranged to [plane, quarter, ci,
34, 130] fp32 — the H dim split into 4 quarter-row panels of 32 rows,
each padded with 1 halo row above/below and 1 zero col left/right, so
the device needs no data reshuffling at all.

Per-core kernel (Bass/Tile):
  - One [128, 34*130] DMA per plane (partition = quarter*32 + ci), then
    ScalarE Sign fp32 -> bf16 (zero pads stay zero).
  - Conv = 27 accumulating matmuls (K=32 ci, M=64 co, N=512) per 4-row
    output tile; every (kd, kh, kw) tap is a free-dim AP offset.
  - 16-way PE tiling: quarter q runs on PE row-group q (tile_position
    row 32q); even/odd 4-row blocks run on PE column halves. 8 matmuls
    issue back-to-back per tap and run concurrently: full 128x128 array.
  - PSUM: 4 banks per generation (bank = quarter, partitions 0-63 even
    block / 64-127 odd block), double-buffered = all 8 banks.
  - PSUM drained with bias add on ScalarE (even) / VectorE (odd) into a
    [128, 2048] staging tile, flushed to HBM as 2x512KB DMAs on
    complementary SBUF port sets.
"""

import numpy as np
import ml_dtypes

import concourse.bass as bass
import concourse.mybir as mybir
import concourse.tile as tile
from concourse import bacc
from concourse.bass import ts
from concourse.bass_utils import run_bass_kernel_spmd
from concourse.tile_rust import add_dep_helper

CI = 32
CO = 64
D_FULL = 64
N_CORES = 8
D_OUT = D_FULL // N_CORES  # output planes per core
D_IN = D_OUT + 2  # input planes per core (1-plane halo each side)

_cache = {}


def build_conv_program(n_in_planes=D_IN, n_out_planes=D_OUT, H=128, W=128,
                       debug=False):
    """Build the per-core Bass program (SPMD: same program on all cores)."""
    f32 = mybir.dt.float32
    bf16 = mybir.dt.bfloat16
    Hq = H // 4          # rows per quarter-panel
    Hqp, Wp = Hq + 2, W + 2
    n_pairs = Hq // 8    # even/odd block pairs per quarter
    assert Hq % 8 == 0 and W == 128

    nc = bacc.Bacc("TRN2", target_bir_lowering=False, debug=debug)
    x_in = nc.declare_dram_parameter(
        "xs", [n_in_planes, 4, CI, Hqp, Wp], f32, isOutput=False)
    w_in = nc.declare_dram_parameter("wst", [128, 27, 2 * CO], bf16,
                                     isOutput=False)
    b_in = nc.declare_dram_parameter("bias", [128, 1], f32, isOutput=False)
    y_out = nc.declare_dram_parameter("y", [CO, n_out_planes, H, W], f32,
                                      isOutput=True)

    with tile.TileContext(nc) as tc:
        with (
            tc.tile_pool(name="const", bufs=1) as constp,
            tc.tile_pool(name="raw", bufs=2) as rawp,
            tc.tile_pool(name="sgn", bufs=5) as sgnp,
            tc.tile_pool(name="stg", bufs=4) as stgp,
            tc.tile_pool(name="psum", bufs=2, space="PSUM") as psump,
        ):
            wt = constp.tile([128, 27, 2 * CO], bf16)
            nc.sync.dma_start(out=wt[:], in_=w_in[:])
            bs = constp.tile([128, 1], f32)
            nc.sync.dma_start(out=bs[:], in_=b_in[:])

            sgns = {}

            def load_plane(p):
                raw = rawp.tile([128, Hqp, Wp], f32, tag="raw")
                nc.sync.dma_start(
                    out=raw[:],
                    in_=x_in[p].rearrange("q c h w -> (q c) h w"),
                )
                sgn = sgnp.tile([128, Hqp, Wp], bf16, tag="sgn")
                nc.scalar.sign(sgn[:], raw[:])
                sgns[p] = sgn

            for p in range(3):
                load_plane(p)

            for d in range(n_out_planes):
                if d + 3 < n_in_planes:
                    load_plane(d + 3)
                for pi in range(n_pairs):
                    # generation: for each quarter, blocks 2*pi (even,
                    # cols 0-63) and 2*pi+1 (odd, cols 64-127)
                    pts = [psump.tile([128, 512], f32, tag=f"pt{q}",
                                      name=f"pt{q}_{d}_{pi}")
                           for q in range(4)]
                    for tap in range(27):
                        kd, r = divmod(tap, 9)
                        kh, kw = divmod(r, 3)
                        sg = sgns[d + kd]
                        for q in range(4):
                            for half in range(2):
                                blk = 2 * pi + half
                                rhs = sg[32 * q:32 * q + 32,
                                         4 * blk + kh:4 * blk + kh + 4,
                                         kw:kw + W]
                                nc.tensor.matmul(
                                    pts[q][64 * half:64 * half + 64, :],
                                    lhsT=wt[32 * q:32 * q + 32, tap, 0:CO],
                                    rhs=rhs,
                                    start=(tap == 0),
                                    stop=(tap == 26),
                                    tile_position=(32 * q, 64 * half),
                                    skip_group_check=True,
                                )
                    # drain with bias add, split across ScalarE and VectorE
                    stg = stgp.tile([128, 4 * 512], f32, tag="stg")
                    for q in range(4):
                        nc.scalar.activation(
                            stg[0:64, ts(q, 512)], pts[q][0:64, :],
                            mybir.ActivationFunctionType.Identity,
                            bias=bs[0:64], scale=1.0,
                        )
                        nc.vector.tensor_scalar_add(
                            out=stg[64:128, ts(q, 512)], in0=pts[q][64:128, :],
                            scalar1=bs[64:128],
                        )
                    # out rows: quarter q -> 32q + 8*pi + 4*half .. +4
                    yv = y_out[:, d].rearrange("co (q hi) w -> co q hi w", q=4)
                    for half in range(2):
                        dst = yv[:, :, 8 * pi + 4 * half:8 * pi + 4 * half + 4,
                                 :].rearrange("co q hi w -> co q (hi w)")
                        src = stg[64 * half:64 * half + 64, :].rearrange(
                            "co (q n) -> co q n", q=4)
                        nc.sync.dma_start(out=dst, in_=src)

    nc.compile()
    return nc


def _batch_matmul_sem_incs(nc):
    """Replace per-matmul semaphore increments with batched increments on
    threshold-crossing matmuls only.

    Concurrent MATMULs on the PE complete in pc order, so a single +N inc
    on matmul #k is equivalent to +1 incs on matmuls #1..#k for every
    waiter. Sem-write serialization costs ~26ns per inc on the PE, which
    at ~7000 matmuls dominates issue bandwidth. Only sems whose updaters
    are exclusively immediate-mode matmul incs are touched; every waited
    value (ge or eq, immediate only) is preserved exactly.
    """
    import bisect
    from collections import defaultdict

    f = nc.m.functions[0]
    insts = [i for blk in f.blocks for i in blk.instructions]
    updaters = defaultdict(list)  # sem id -> [(order, inst, value)]
    unsafe = set()  # sem ids we must not touch
    wait_vals = defaultdict(set)
    for inst in insts:
        si = inst.sync_info
        if si is None:
            continue
        for u in si.on_update:
            if u.sync_type != 'semaphore':
                continue
            if (isinstance(inst, mybir.InstMatmult)
                    and u.update_mode == 'sem-inc'
                    and u.update_reg is None):
                updaters[u.id].append((len(updaters[u.id]), inst, u.update_value))
            else:
                unsafe.add(u.id)
        for w in si.on_wait:
            if w.sync_type != 'semaphore':
                continue
            if w.wait_reg is not None or w.wait_value is None:
                unsafe.add(w.id)
            else:
                wait_vals[w.id].add(w.wait_value)

    n_removed = 0
    for sem, ups in updaters.items():
        if sem in unsafe or len(ups) < 16:
            continue
        if any(v != 1 for _, _, v in ups):
            continue
        total = len(ups)
        if any(v > total for v in wait_vals.get(sem, set())):
            continue
        # marks: the minimal updater indices that waiters actually need
        # (plus the final one, so sem totals seen by teardown code move
        # monotonically to a deterministic value)
        need = sorted(wait_vals.get(sem, set()) | {total})
        mark_ks = sorted({wv - 1 for wv in need})  # 0-based updater index
        # rank map: old threshold w -> 1 + index of first mark with
        # cumulative (= k+1) >= w
        cums = [k + 1 for k in mark_ks]
        mark_set = set(mark_ks)
        # strip +1 updates from unmarked matmuls (engine completes in pc
        # order, so the marked matmul's inc implies all earlier ones)
        for k, (_, inst, _) in enumerate(ups):
            if k in mark_set:
                continue
            si = inst.sync_info
            si.on_update = [u for u in si.on_update
                            if not (u.sync_type == 'semaphore'
                                    and u.id == sem)]
            inst.sync_info = si
            n_removed += 1
        # rewrite every wait on this sem into mark-rank space
        import bass_rust
        for inst in insts:
            si = inst.sync_info
            if si is None or not si.on_wait:
                continue
            changed = False
            lst = []
            for w in si.on_wait:
                if w.sync_type == 'semaphore' and w.id == sem:
                    nv = bisect.bisect_left(cums, w.wait_value) + 1
                    lst.append(bass_rust.SyncWait(
                        sync_type=w.sync_type, id=w.id, ant_name=w.ant_name,
                        wait_mode=w.wait_mode, wait_value=nv,
                        wait_reg=w.wait_reg))
                    changed = True
                else:
                    lst.append(w)
            if changed:
                si.on_wait = lst
                inst.sync_info = si
    return n_removed


def _get_program():
    if "nc" not in _cache:
        _cache["nc"] = build_conv_program()
    return _cache["nc"]


def prep_weights(W, b):
    W = np.asarray(W, dtype=np.float32)
    b = np.asarray(b, dtype=np.float32)
    # wst[q*32+ci, kd*9+kh*3+kw, half*64+co] = W[co, ci, kd, kh, kw],
    # replicated over the 4 row groups and the 2 col halves
    wq = W.transpose(1, 2, 3, 4, 0).reshape(CI, 27, CO)
    wq2 = np.concatenate([wq, wq], axis=2)  # duplicate col halves
    wst = np.ascontiguousarray(
        np.broadcast_to(wq2[None], (4, CI, 27, 2 * CO)).reshape(128, 27, 2 * CO)
    ).astype(ml_dtypes.bfloat16)
    bias = np.ascontiguousarray(
        np.concatenate([b, b]).reshape(128, 1).astype(np.float32))
    return wst, bias


def prep_x_slab(xpad, p_lo, n_planes, H=128, W=128):
    """xpad: [CI, D+2, H+2, W+2] zero-padded input. Returns
    [n_planes, 4, CI, H//4+2, W+2] fp32 slab for planes p_lo..p_lo+n_planes."""
    Hq = H // 4
    out = np.empty((n_planes, 4, CI, Hq + 2, W + 2), dtype=np.float32)
    for q in range(4):
        # padded rows 32q .. 32q+34 cover global rows 32q-1 .. 32q+33
        out[:, q] = xpad[:, p_lo:p_lo + n_planes,
                         Hq * q:Hq * q + Hq + 2, :].transpose(1, 0, 2, 3)
    return out


def _prep_inputs(x, W, b):
    x = np.asarray(x, dtype=np.float32)
    wst, bias = prep_weights(W, b)
    xpad = np.pad(x[0], ((0, 0), (1, 1), (1, 1), (1, 1)))
    in_maps = []
    for k in range(N_CORES):
        xs = prep_x_slab(xpad, D_OUT * k, D_IN)
        in_maps.append({"xs": xs, "wst": wst, "bias": bias})
    return in_maps


def run(x, W, b, trace=False):
    """Run the kernel; returns (output, BassKernelResults)."""
    nc = _get_program()
    in_maps = _prep_inputs(x, W, b)
    res = run_bass_kernel_spmd(nc, in_maps, list(range(N_CORES)), trace=trace)
    y = np.concatenate([res.results[k]["y"] for k in range(N_CORES)], axis=1)
    return y[None], res


def kernel(x, W, b):
    y, _ = run(x, W, b)
    return y


# revision 31
# speedup vs baseline: 1.0566x; 1.0385x over previous
"""BinConv3d (sign-binarized 3x3x3 conv, NCDHW) on 8 Trainium2 NeuronCores.

Full inputs in, full output out. Sharding: depth dim D=64 split 8 ways
(8 output planes per core) with a 1-plane halo on the input; conv weights
and bias replicated to every core.

Host prep: each core's input slab is rearranged to [plane, quarter, ci,
34, 130] fp32 — the H dim split into 4 quarter-row panels of 32 rows,
each padded with 1 halo row above/below and 1 zero col left/right, so
the device needs no data reshuffling at all.

Per-core kernel (Bass/Tile):
  - One [128, 34*130] DMA per plane (partition = quarter*32 + ci), then
    ScalarE Sign fp32 -> bf16 (zero pads stay zero).
  - Conv = 27 accumulating matmuls (K=32 ci, M=64 co, N=512) per 4-row
    output tile; every (kd, kh, kw) tap is a free-dim AP offset.
  - 16-way PE tiling: quarter q runs on PE row-group q (tile_position
    row 32q); even/odd 4-row blocks run on PE column halves. 8 matmuls
    issue back-to-back per tap and run concurrently: full 128x128 array.
  - PSUM: 4 banks per generation (bank = quarter, partitions 0-63 even
    block / 64-127 odd block), double-buffered = all 8 banks.
  - PSUM drained with bias add on ScalarE (even) / VectorE (odd) into a
    [128, 2048] staging tile, flushed to HBM as 2x512KB DMAs on
    complementary SBUF port sets.
"""

import numpy as np
import ml_dtypes

import concourse.bass as bass
import concourse.mybir as mybir
import concourse.tile as tile
from concourse import bacc
from concourse.bass import ts
from concourse.bass_utils import run_bass_kernel_spmd

CI = 32
CO = 64
D_FULL = 64
N_CORES = 8
D_OUT = D_FULL // N_CORES  # output planes per core
D_IN = D_OUT + 2  # input planes per core (1-plane halo each side)

_cache = {}


def build_conv_program(n_in_planes=D_IN, n_out_planes=D_OUT, H=128, W=128,
                       debug=False):
    """Build the per-core Bass program (SPMD: same program on all cores)."""
    f32 = mybir.dt.float32
    bf16 = mybir.dt.bfloat16
    Hq = H // 4          # rows per quarter-panel
    Hqp, Wp = Hq + 2, W + 2
    n_pairs = Hq // 8    # even/odd block pairs per quarter
    assert Hq % 8 == 0 and W == 128

    nc = bacc.Bacc("TRN2", target_bir_lowering=False, debug=debug)
    x_in = nc.declare_dram_parameter(
        "xs", [n_in_planes, 4, CI, Hqp, Wp], f32, isOutput=False)
    w_in = nc.declare_dram_parameter("wst", [128, 27, 2 * CO], bf16,
                                     isOutput=False)
    b_in = nc.declare_dram_parameter("bias", [128, 1], f32, isOutput=False)
    y_out = nc.declare_dram_parameter("y", [CO, n_out_planes, H, W], f32,
                                      isOutput=True)

    with tile.TileContext(nc) as tc:
        with (
            tc.tile_pool(name="const", bufs=1) as constp,
            tc.tile_pool(name="raw", bufs=2) as rawp,
            tc.tile_pool(name="sgn", bufs=5) as sgnp,
            tc.tile_pool(name="stg", bufs=4) as stgp,
            tc.tile_pool(name="psum", bufs=2, space="PSUM") as psump,
        ):
            wt = constp.tile([128, 27, 2 * CO], bf16)
            nc.sync.dma_start(out=wt[:], in_=w_in[:])
            bs = constp.tile([128, 1], f32)
            nc.sync.dma_start(out=bs[:], in_=b_in[:])

            sgns = {}
            # panel rows are loaded in two halves so the first matmuls can
            # start after half of planes 0-2 arrived: top covers panel rows
            # [0, Hh+2) (blocks 0..Hq/8-1), bottom rows [Hh, Hqp)
            Hh = Hq // 2

            def load_half(p, lo, n):
                raw = rawp.tile([128, n, Wp], f32, tag="raw")
                nc.sync.dma_start(
                    out=raw[:],
                    in_=x_in[p, :, :, lo:lo + n].rearrange(
                        "q c h w -> (q c) h w"),
                )
                sgn = sgnp.tile([128, n, Wp], bf16, tag="sgn")
                nc.scalar.sign(sgn[:], raw[:])
                return sgn

            def load_plane(p, part):
                if part == 0:
                    sgns[p] = [load_half(p, 0, Hh + 2), None]
                else:
                    sgns[p][1] = load_half(p, Hh, Hh + 2)

            for p in range(3):
                load_plane(p, 0)
            for p in range(3):
                load_plane(p, 1)

            for d in range(n_out_planes):
                if d + 3 < n_in_planes:
                    load_plane(d + 3, 0)
                    load_plane(d + 3, 1)
                for pi in range(n_pairs):
                    # generation: for each quarter, blocks 2*pi (even,
                    # cols 0-63) and 2*pi+1 (odd, cols 64-127)
                    pts = [psump.tile([128, 512], f32, tag=f"pt{q}",
                                      name=f"pt{q}_{d}_{pi}")
                           for q in range(4)]
                    for tap in range(27):
                        kd, r = divmod(tap, 9)
                        kh, kw = divmod(r, 3)
                        for q in range(4):
                            for half in range(2):
                                blk = 2 * pi + half
                                top = blk < Hq // 8
                                sg = sgns[d + kd][0 if top else 1]
                                row = 4 * blk + kh - (0 if top else Hh)
                                rhs = sg[32 * q:32 * q + 32,
                                         row:row + 4,
                                         kw:kw + W]
                                nc.tensor.matmul(
                                    pts[q][64 * half:64 * half + 64, :],
                                    lhsT=wt[32 * q:32 * q + 32, tap, 0:CO],
                                    rhs=rhs,
                                    start=(tap == 0),
                                    stop=(tap == 26),
                                    tile_position=(32 * q, 64 * half),
                                    skip_group_check=True,
                                )
                    # drain with bias add, split across ScalarE and VectorE
                    stg = stgp.tile([128, 4 * 512], f32, tag="stg")
                    for q in range(4):
                        nc.scalar.activation(
                            stg[0:64, ts(q, 512)], pts[q][0:64, :],
                            mybir.ActivationFunctionType.Identity,
                            bias=bs[0:64], scale=1.0,
                        )
                        nc.vector.tensor_scalar_add(
                            out=stg[64:128, ts(q, 512)], in0=pts[q][64:128, :],
                            scalar1=bs[64:128],
                        )
                    # out rows: quarter q -> 32q + 8*pi + 4*half .. +4
                    yv = y_out[:, d].rearrange("co (q hi) w -> co q hi w", q=4)
                    for half in range(2):
                        dst = yv[:, :, 8 * pi + 4 * half:8 * pi + 4 * half + 4,
                                 :].rearrange("co q hi w -> co q (hi w)")
                        src = stg[64 * half:64 * half + 64, :].rearrange(
                            "co (q n) -> co q n", q=4)
                        nc.sync.dma_start(out=dst, in_=src)

    nc.compile()
    return nc


def _get_program():
    if "nc" not in _cache:
        _cache["nc"] = build_conv_program()
    return _cache["nc"]


def prep_weights(W, b):
    W = np.asarray(W, dtype=np.float32)
    b = np.asarray(b, dtype=np.float32)
    # wst[q*32+ci, kd*9+kh*3+kw, half*64+co] = W[co, ci, kd, kh, kw],
    # replicated over the 4 row groups and the 2 col halves
    wq = W.transpose(1, 2, 3, 4, 0).reshape(CI, 27, CO)
    wq2 = np.concatenate([wq, wq], axis=2)  # duplicate col halves
    wst = np.ascontiguousarray(
        np.broadcast_to(wq2[None], (4, CI, 27, 2 * CO)).reshape(128, 27, 2 * CO)
    ).astype(ml_dtypes.bfloat16)
    bias = np.ascontiguousarray(
        np.concatenate([b, b]).reshape(128, 1).astype(np.float32))
    return wst, bias


def prep_x_slab(xpad, p_lo, n_planes, H=128, W=128):
    """xpad: [CI, D+2, H+2, W+2] zero-padded input. Returns
    [n_planes, 4, CI, H//4+2, W+2] fp32 slab for planes p_lo..p_lo+n_planes."""
    Hq = H // 4
    out = np.empty((n_planes, 4, CI, Hq + 2, W + 2), dtype=np.float32)
    for q in range(4):
        # padded rows 32q .. 32q+34 cover global rows 32q-1 .. 32q+33
        out[:, q] = xpad[:, p_lo:p_lo + n_planes,
                         Hq * q:Hq * q + Hq + 2, :].transpose(1, 0, 2, 3)
    return out


def _prep_inputs(x, W, b):
    x = np.asarray(x, dtype=np.float32)
    wst, bias = prep_weights(W, b)
    xpad = np.pad(x[0], ((0, 0), (1, 1), (1, 1), (1, 1)))
    in_maps = []
    for k in range(N_CORES):
        xs = prep_x_slab(xpad, D_OUT * k, D_IN)
        in_maps.append({"xs": xs, "wst": wst, "bias": bias})
    return in_maps


def run(x, W, b, trace=False):
    """Run the kernel; returns (output, BassKernelResults)."""
    nc = _get_program()
    in_maps = _prep_inputs(x, W, b)
    res = run_bass_kernel_spmd(nc, in_maps, list(range(N_CORES)), trace=trace)
    y = np.concatenate([res.results[k]["y"] for k in range(N_CORES)], axis=1)
    return y[None], res


def kernel(x, W, b):
    y, _ = run(x, W, b)
    return y
